# revision 42
# baseline (speedup 1.0000x reference)
"""Trainium2 Bass kernel for a 3-layer GCN encoder (PyG GCNConv x3 + global mean pool).

Strategy (8 NeuronCores, v2):
  - Nodes sharded contiguously (6250/core, padded to 6272 = 49 blocks of 128);
    edges partitioned by destination, bucketed per (dst-block, src-segment).
  - Per layer k:  out = A_hat @ (g @ W) + b  with g = dinv * h, reassociated as
    (A_hat @ g) @ W.  Per dst block:
        agg[f, slot] = sum_e g[src_e, f] * OH[e, slot]      (PE, bf16)
                     + g_own[slot -> f] @ I                 (self-loops, PE)
        h' = relu(dinv * (agg @ W) + b)
    The per-edge one-hot OH (ew at [e, slot]) is HOST-precomputed in bf16 and
    streamed from HBM; no on-device one-hot construction at all.
  - Per-edge rows gathered from HBM in bf16 (256 B descriptors) via SWDGE
    dma_gather, 1024 idxs/call, 4 queues, deep staging (measured floor
    ~2.8 ns/descriptor on the Pool engine -- the kernel's critical resource).
  - deg/dinv and g0 = dinv*x are host-side preprocessing (edge metadata and an
    elementwise input scale); all matmuls/aggregation run on device.
  - The gathered-feature table is AllGathered between layers in bf16, split in
    two segments (A: blocks 0-23, B: 24-48) so segment A's collective overlaps
    with the tail half of the layer's compute.  The segment split also keeps
    gather indices within int16 (rows < 32768 per segment table).
  - Final global mean pool: per-core one-hot matmul into [64, 128]; host sums
    the 8 partials and divides by counts.
"""

import os
import sys

import numpy as np
import ml_dtypes

NP_BF16 = ml_dtypes.bfloat16
NP_FP8 = ml_dtypes.float8_e4m3fn

for _p in ("/opt/trn_rl_repo",):
    if _p not in sys.path and os.path.isdir(_p):
        sys.path.insert(0, _p)

import concourse.bass as bass
import concourse.bacc as bacc
import concourse.tile as tile
import concourse.mybir as mybir
from concourse import bass_utils
from concourse.alu_op_type import AluOpType

F32 = mybir.dt.float32
BF16 = mybir.dt.bfloat16
FP8 = mybir.dt.float8e4
I16 = mybir.dt.int16
AF = mybir.ActivationFunctionType

GMAX = 8       # max chunks (of 128 idxs) per dma_gather call (ring limit 1024)
NQ = 4         # SWDGE queues
GRP = 8        # dst blocks per gather/compute group


class Cfg:
    def __init__(self, n_nodes=50000, n_cores=8, d=128, n_graphs=64):
        self.n_nodes = n_nodes
        self.n_cores = n_cores
        self.d = d
        self.n_graphs = n_graphs
        self.shard = n_nodes // n_cores          # 6250
        self.n_blk = (self.shard + 127) // 128   # 49
        self.shard_p = self.n_blk * 128          # 6272
        # node segments (per-core row ranges); each segment's global table
        # stays < 32768 rows (int16 gather indices), and the segments' last
        # blocks are spread out so each AllGather can start early and the
        # layer-boundary segments are small
        self.seg_base = [0, 2560, 5120, 5888]
        self.seg_end = [2560, 5120, 5888, 6272]
        self.seg_rows = [e - b for b, e in zip(self.seg_base, self.seg_end)]
        self.n_seg = len(self.seg_base)
        self.seg_last_blk = [e // 128 - 1 for e in self.seg_end]
        self.n_grp = (self.n_blk + GRP - 1) // GRP


def bucket_order(cfg):
    """Stream order of (block, segment) buckets: (group, seg, block)."""
    order = []
    for g in range(cfg.n_grp):
        bs = range(g * GRP, min(cfg.n_blk, (g + 1) * GRP))
        for h in range(cfg.n_seg):
            for b in bs:
                order.append((b, h))
    return order


def preprocess(cfg, edge_index, edge_weight, x, batch):
    src = np.asarray(edge_index)[0].astype(np.int64)
    dst = np.asarray(edge_index)[1].astype(np.int64)
    ew = np.asarray(edge_weight).astype(np.float32)
    n, C, S = cfg.n_nodes, cfg.n_cores, cfg.shard
    ne = src.shape[0]

    deg = np.bincount(dst, weights=ew, minlength=n) + 1.0
    dinv = (1.0 / np.sqrt(deg)).astype(np.float32)     # [n]
    g0 = (np.asarray(x, np.float32) * dinv[:, None])   # [n, d] fp32

    # destination decomposition
    core = dst // S
    l = dst - core * S
    b = l // 128
    slot = l - b * 128
    # source -> (segment, row) in the segment tables
    sc = src // S
    r = src - sc * S
    half = np.digitize(r, cfg.seg_base[1:])        # segment id 0..n_seg-1
    sbase = np.array(cfg.seg_base)[half]
    srows = np.array(cfg.seg_rows)[half]
    row = sc * srows + (r - sbase)

    order = bucket_order(cfg)
    ns = cfg.n_seg
    bpos = np.zeros(cfg.n_blk * ns, np.int64)
    for i, (bb, hh) in enumerate(order):
        bpos[bb * ns + hh] = i
    skey = bpos[b * ns + half]                     # bucket stream index
    key = core * len(order) + skey

    # Dedupe (src -> dst-block): duplicate edges share one gathered row;
    # their one-hot row then carries multiple nonzeros (one per dst slot).
    osort = np.lexsort((row, key))
    keyo, rowo = key[osort], row[osort]
    lead = np.ones(ne, bool)
    lead[1:] = (keyo[1:] != keyo[:-1]) | (rowo[1:] != rowo[:-1])
    group = np.cumsum(lead) - 1                    # per sorted edge
    nl = int(lead.sum())
    lkey = keyo[lead]                              # bucket key per group

    cnt = np.bincount(lkey, minlength=C * len(order))
    cnt2 = cnt.reshape(C, len(order))
    seg = ((cnt2.max(axis=0) + 127) // 128) * 128   # [n_buckets] stream order
    seg_off = np.concatenate([[0], np.cumsum(seg)])
    ep = int(seg_off[-1])

    starts = np.concatenate([[0], np.cumsum(cnt)])[:-1]
    rank = np.arange(nl) - starts[lkey]
    gpos = seg_off[lkey % len(order)] + rank       # slot per unique group
    pos = gpos[group]                              # per sorted edge
    core_s = keyo // len(order)

    idx16 = np.zeros((C, ep), np.int16)
    idx16[core_s[lead], gpos] = rowo[lead].astype(np.int16)
    nchunk = ep // 128
    oh = np.zeros((C, 128, nchunk * 128), np.float32)
    np.add.at(oh, (core_s, pos % 128, (pos // 128) * 128 + slot[osort]),
              ew[osort])
    oh = oh.astype(NP_FP8)

    idxw = idx16.reshape(C, ep // 16, 16).transpose(0, 2, 1)
    idxw = np.tile(idxw, (1, 8, 1)).copy()          # [C, 128, ep//16]

    # per-core aux arrays: gown0[c, l%128, (l//128)*128 + f] = g0[c*S+l, f]
    batch = np.asarray(batch).astype(np.int64)
    lr = np.arange(S)
    li = np.tile(lr, C)
    ci = np.repeat(np.arange(C), S)
    gown0 = np.zeros((C, 128, cfg.n_blk * 128), np.float32)
    gown0[ci[:, None], (li % 128)[:, None],
          ((li // 128) * 128)[:, None] + np.arange(cfg.d)[None, :]] = g0[
        ci * S + li]
    dinvb = np.zeros((C, 128, cfg.n_blk), np.float32)
    dinvb[ci, li % 128, li // 128] = dinv[ci * S + li]
    poolm = np.zeros((C, 128, cfg.n_blk * cfg.n_graphs), np.float32)
    poolm[ci, li % 128, (li // 128) * cfg.n_graphs + batch[ci * S + li]] = 1.0

    # layer-0 segment tables (padded local rows are zero)
    gfs0 = []
    for s in range(cfg.n_seg):
        rows = cfg.seg_rows[s]
        t = np.zeros((C * rows, cfg.d), np.float32)
        rs = lr[(lr >= cfg.seg_base[s]) & (lr < cfg.seg_end[s])]
        for c in range(C):
            t[c * rows + (rs - cfg.seg_base[s])] = g0[c * S + rs]
        gfs0.append(t.astype(NP_BF16))

    counts = np.bincount(batch, minlength=cfg.n_graphs).astype(np.float32)
    return dict(seg=seg, seg_off=seg_off, ep=ep, nchunk=nchunk,
                idxw=idxw, oh=oh,
                gown0=gown0.astype(NP_BF16), dinvb=dinvb,
                poolm=poolm.astype(NP_BF16), gfs0=gfs0,
                counts=counts)


def build_program(cfg, seg, seg_off, ep):
    """SPMD Bass/Tile program; trip counts depend only on seg (shared)."""
    d, g64, n_blk = cfg.d, cfg.n_graphs, cfg.n_blk
    order = bucket_order(cfg)
    nchunk = ep // 128
    # chunk ranges per bucket (stream order)
    boff = {order[i]: int(seg_off[i]) // 128 for i in range(len(order))}
    bcnt = {order[i]: int(seg[i]) // 128 for i in range(len(order))}

    nc = bacc.Bacc("TRN2", target_bir_lowering=False, debug=False,
                   enable_asserts=False, num_devices=cfg.n_cores,
                   num_swdge_queues=NQ)

    gfs_in = [nc.dram_tensor(f"gfs0_{s}",
                             [cfg.n_cores * cfg.seg_rows[s], d], BF16,
                             kind="ExternalInput") for s in range(cfg.n_seg)]
    gown_in = nc.dram_tensor("gown0", [128, n_blk * 128], BF16,
                             kind="ExternalInput")
    idx_in = nc.dram_tensor("idxw", [128, ep // 16], I16, kind="ExternalInput")
    oh_in = nc.dram_tensor("oh", [128, nchunk * 128], FP8,
                           kind="ExternalInput")
    poolm_in = nc.dram_tensor("poolm", [128, n_blk * g64], BF16,
                              kind="ExternalInput")
    dinv_in = nc.dram_tensor("dinvb", [128, n_blk], F32, kind="ExternalInput")
    w_in = nc.dram_tensor("wmats", [3, d, d], BF16, kind="ExternalInput")
    b_in = nc.dram_tensor("biasb", [3, 128, d], F32, kind="ExternalInput")
    id_in = nc.dram_tensor("ident", [128, 128], BF16, kind="ExternalInput")
    out_t = nc.dram_tensor("pool_out", [g64, d], F32, kind="ExternalOutput")

    g_loc = [[nc.dram_tensor(f"g_loc{k}_{s}", [cfg.seg_rows[s], d], BF16,
                             kind="Internal") for s in range(cfg.n_seg)]
             for k in (1, 2)]
    gfs = [[nc.dram_tensor(f"gfs{k}_{s}",
                           [cfg.n_cores * cfg.seg_rows[s], d], BF16,
                           kind="Internal", addr_space="Shared")
            for s in range(cfg.n_seg)] for k in (1, 2)]
    rg = [list(range(cfg.n_cores))]
    # Collective schedule.  Segment s's AllGather must be EMITTED after the
    # block loop that writes its last g_loc rows; beyond that it is deferred
    # (emission point = after the NEXT group's seg-0 gather calls) so the
    # in-order Pool queue overlaps the semaphore wait with useful desc-gen.
    # Segments finishing in the final group are emitted inside the next
    # layer's first group (their data is only needed by its seg>=1 calls).
    ag_sched = {}   # group index -> segments, emitted after group's seg-0
    ag_tail = []    # deferred into the next layer's group 0
    for s in range(cfg.n_seg):
        last_grp = cfg.seg_last_blk[s] // GRP
        if last_grp + 1 >= cfg.n_grp:
            ag_tail.append(s)
        else:
            ag_sched.setdefault(last_grp + 1, []).append(s)

    with tile.TileContext(nc) as tc:
        with tc.tile_pool(name="const", bufs=1) as cp:
            idx_sb = cp.tile([128, ep // 16], I16, tag="idx")
            # split the idx load so the first group's gathers start sooner
            c_sp = max(8, (ep // 16) // 8)
            nc.sync.dma_start(idx_sb[:, :c_sp], idx_in.ap()[:, :c_sp])
            nc.sync.dma_start(idx_sb[:, c_sp:], idx_in.ap()[:, c_sp:])
            poolm_sb = cp.tile([128, n_blk * g64], BF16, tag="poolm")
            nc.sync.dma_start(poolm_sb[:, :], poolm_in.ap())
            dinv_sb = cp.tile([128, n_blk], F32, tag="dinv")
            nc.sync.dma_start(dinv_sb[:, :], dinv_in.ap())
            i128_sb = cp.tile([128, 128], BF16, tag="i128")
            nc.sync.dma_start(i128_sb[:, :], id_in.ap())
            w_sb, b_sb = [], []
            for k in range(3):
                wt = cp.tile([d, d], BF16, tag=f"w{k}", name=f"w{k}")
                nc.sync.dma_start(wt[:, :], w_in.ap()[k, :, :])
                w_sb.append(wt)
                bt = cp.tile([128, d], F32, tag=f"b{k}", name=f"b{k}")
                nc.sync.dma_start(bt[:, :], b_in.ap()[k, :, :])
                b_sb.append(bt)
            gpp = [[cp.tile([128, 128], BF16, tag=f"gown{i}_{b}",
                            name=f"gown{i}_{b}") for b in range(n_blk)]
                   for i in (0, 1)]
            for b in range(n_blk):
                nc.sync.dma_start(gpp[0][b][:, :],
                                  gown_in.ap()[:, b * 128:(b + 1) * 128])

            with tc.tile_pool(name="stage", bufs=30) as stp, \
                 tc.tile_pool(name="ohp", bufs=3) as ohp, \
                 tc.tile_pool(name="aggp", bufs=5, space="PSUM") as psA, \
                 tc.tile_pool(name="outp", bufs=2, space="PSUM") as psB, \
                 tc.tile_pool(name="poolp", bufs=1, space="PSUM") as psC, \
                 tc.tile_pool(name="work", bufs=6) as wp:
                qrr = [0]
                pp = None
                pending_ags = []
                for k in range(3):
                    tbl = [gfs_in[s].ap() if k == 0 else gfs[k - 1][s].ap()
                           for s in range(cfg.n_seg)]
                    gcur = gpp[k % 2]
                    gnxt = gpp[(k + 1) % 2]
                    for g in range(cfg.n_grp):
                        bs = range(g * GRP, min(n_blk, (g + 1) * GRP))
                        chunkmap = {}
                        for h in range(cfg.n_seg):
                            if h == 1:
                                # deferred collectives: emitted once this
                                # group's seg-0 calls are queued so the Pool
                                # queue overlaps their semaphore wait
                                if g == 0 and pending_ags:
                                    for kk, s in pending_ags:
                                        nc.gpsimd.collective_compute(
                                            "AllGather", AluOpType.bypass,
                                            replica_groups=rg,
                                            ins=[g_loc[kk][s].ap()],
                                            outs=[gfs[kk][s].ap()])
                                    pending_ags = []
                                if k < 2:
                                    for s in ag_sched.get(g, []):
                                        nc.gpsimd.collective_compute(
                                            "AllGather", AluOpType.bypass,
                                            replica_groups=rg,
                                            ins=[g_loc[k][s].ap()],
                                            outs=[gfs[k][s].ap()])
                            c0 = boff[(bs[0], h)]
                            c1 = boff[(bs[-1], h)] + bcnt[(bs[-1], h)]
                            base = tbl[h]
                            for cc in range(c0, c1, GMAX):
                                ncall = min(GMAX, c1 - cc)
                                st = stp.tile([128, ncall, 128], BF16,
                                              tag="st",
                                              name=f"st{k}_{g}_{h}_{cc}")
                                nc.gpsimd.dma_gather(
                                    st[:, :, :], base,
                                    idx_sb[:, cc * 8:(cc + ncall) * 8],
                                    ncall * 128, ncall * 128, d,
                                    queue_num=qrr[0] % NQ)
                                qrr[0] += 1
                                for j in range(ncall):
                                    chunkmap[cc + j] = (st, j)
                        for b in bs:
                            nch = sum(bcnt[(b, h)] for h in range(cfg.n_seg))
                            ohb = ohp.tile([128, nch * 128], FP8, tag="ohb",
                                           name=f"oh{k}_{b}")
                            o = 0
                            for h in range(cfg.n_seg):
                                nh = bcnt[(b, h)]
                                if nh == 0:
                                    continue
                                nc.sync.dma_start(
                                    ohb[:, o * 128:(o + nh) * 128],
                                    oh_in.ap()[:, boff[(b, h)] * 128:
                                               (boff[(b, h)] + nh) * 128])
                                o += nh
                            pagg = psA.tile([128, 128], F32, tag="agg",
                                            name=f"agg{k}_{b}")
                            j = 0
                            for h in range(cfg.n_seg):
                                for i in range(bcnt[(b, h)]):
                                    st, jj = chunkmap[boff[(b, h)] + i]
                                    nc.tensor.matmul(
                                        pagg[:, :], st[:, jj, :],
                                        ohb[:, j * 128:(j + 1) * 128],
                                        start=(j == 0), stop=False)
                                    j += 1
                            nc.tensor.matmul(
                                pagg[:, :], gcur[b][:, :],
                                i128_sb[:, :], start=(j == 0), stop=True)
                            aggT = wp.tile([128, 128], BF16, tag="aggT",
                                           name=f"aggT{k}_{b}")
                            nc.scalar.copy(aggT[:, :], pagg[:, :])
                            pout = psB.tile([128, d], F32, tag="out",
                                            name=f"out{k}_{b}")
                            nc.tensor.matmul(pout[:, :], aggT[:, :],
                                             w_sb[k][:, :], start=True,
                                             stop=True)
                            t2 = wp.tile([128, d], BF16, tag="t2",
                                         name=f"t2{k}_{b}")
                            nc.vector.scalar_tensor_tensor(
                                t2[:, :], pout[:, :], dinv_sb[:, b:b + 1],
                                b_sb[k][:, :], AluOpType.mult, AluOpType.add)
                            if k < 2:
                                # g_next = dinv*relu(t2) = relu(dinv*t2)
                                gt = gnxt[b]
                                nc.scalar.activation(
                                    gt[:, :], t2[:, :], AF.Relu,
                                    scale=dinv_sb[:, b:b + 1])
                                s = next(i for i in range(cfg.n_seg)
                                         if b * 128 < cfg.seg_end[i])
                                r0 = b * 128 - cfg.seg_base[s]
                                nc.sync.dma_start(
                                    g_loc[k][s].ap()[r0:r0 + 128, :],
                                    gt[:, :])
                            else:
                                if pp is None:
                                    pp = psC.tile([g64, d], F32, tag="pp")
                                nc.tensor.matmul(
                                    pp[:, :],
                                    poolm_sb[:, b * g64:(b + 1) * g64],
                                    t2[:, :], start=(b == 0),
                                    stop=(b == n_blk - 1))
                    if k < 2:
                        pending_ags += [(k, s) for s in ag_tail]
                ppsb = cp.tile([g64, d], F32, tag="ppsb")
                nc.scalar.copy(ppsb[:, :], pp[:, :])
                nc.sync.dma_start(out_t.ap(), ppsb[:, :])

    nc.compile()
    return nc


def make_in_maps(cfg, prep, ws, bs):
    wmats = np.stack([np.asarray(w, np.float32) for w in ws]).astype(NP_BF16)
    biasb = np.stack([np.broadcast_to(np.asarray(b, np.float32),
                                      (128, cfg.d)) for b in bs]).copy()
    ident = np.eye(128, dtype=np.float32).astype(NP_BF16)
    in_maps = []
    for c in range(cfg.n_cores):
        in_maps.append({
            **{f"gfs0_{s}": prep["gfs0"][s] for s in range(cfg.n_seg)},
            "gown0": prep["gown0"][c], "idxw": prep["idxw"][c],
            "oh": prep["oh"][c], "poolm": prep["poolm"][c],
            "dinvb": prep["dinvb"][c], "wmats": wmats, "biasb": biasb,
            "ident": ident,
        })
    return in_maps


_PROGRAM_CACHE = {}


def run(cfg, x, edge_index, edge_weight, batch, ws, bs, trace=False, trunc=""):
    prep = preprocess(cfg, edge_index, edge_weight, x, batch)
    key = (cfg.n_nodes, cfg.n_cores, prep["ep"], tuple(prep["seg"]))
    nc = _PROGRAM_CACHE.get(key)
    if nc is None:
        nc = build_program(cfg, prep["seg"], prep["seg_off"], prep["ep"])
        _PROGRAM_CACHE[key] = nc
    in_maps = make_in_maps(cfg, prep, ws, bs)
    res = bass_utils.run_bass_kernel_spmd(
        nc, in_maps, core_ids=list(range(cfg.n_cores)), trace=trace)
    partial = np.zeros((cfg.n_graphs, cfg.d), np.float64)
    for c in range(cfg.n_cores):
        partial += res.results[c]["pool_out"].astype(np.float64)
    out = (partial / np.maximum(prep["counts"], 1.0)[:, None]).astype(
        np.float32)
    return out, res


def kernel(x, edge_index, edge_weight, batch, W0, b0, W1, b1, W2, b2):
    cfg = Cfg()
    trace = bool(int(os.environ.get("GCN_TRACE", "0")))
    out, _ = run(cfg, x, edge_index, edge_weight, batch,
                 [W0, W1, W2], [b0, b1, b2], trace=trace)
    return out


# revision 45
# speedup vs baseline: 1.1010x; 1.1010x over previous
"""Trainium2 Bass kernel for a 3-layer GCN encoder (PyG GCNConv x3 + global mean pool).

Strategy (8 NeuronCores, v2):
  - Nodes sharded contiguously (6250/core, padded to 6272 = 49 blocks of 128);
    edges partitioned by destination, bucketed per (dst-block, src-segment).
  - Per layer k:  out = A_hat @ (g @ W) + b  with g = dinv * h, reassociated as
    (A_hat @ g) @ W.  Per dst block:
        agg[f, slot] = sum_e g[src_e, f] * OH[e, slot]      (PE, bf16)
                     + g_own[slot -> f] @ I                 (self-loops, PE)
        h' = relu(dinv * (agg @ W) + b)
    The per-edge one-hot OH (ew at [e, slot]) is HOST-precomputed in bf16 and
    streamed from HBM; no on-device one-hot construction at all.
  - Per-edge rows gathered from HBM in bf16 (256 B descriptors) via SWDGE
    dma_gather, 1024 idxs/call, 4 queues, deep staging (measured floor
    ~2.8 ns/descriptor on the Pool engine -- the kernel's critical resource).
  - deg/dinv and g0 = dinv*x are host-side preprocessing (edge metadata and an
    elementwise input scale); all matmuls/aggregation run on device.
  - The gathered-feature table is AllGathered between layers in bf16, split in
    two segments (A: blocks 0-23, B: 24-48) so segment A's collective overlaps
    with the tail half of the layer's compute.  The segment split also keeps
    gather indices within int16 (rows < 32768 per segment table).
  - Final global mean pool: per-core one-hot matmul into [64, 128]; host sums
    the 8 partials and divides by counts.
"""

import os
import sys

import numpy as np
import ml_dtypes

NP_BF16 = ml_dtypes.bfloat16
NP_FP8 = ml_dtypes.float8_e4m3fn

for _p in ("/opt/trn_rl_repo",):
    if _p not in sys.path and os.path.isdir(_p):
        sys.path.insert(0, _p)

import concourse.bass as bass
import concourse.bacc as bacc
import concourse.tile as tile
import concourse.mybir as mybir
from concourse import bass_utils
from concourse.alu_op_type import AluOpType

F32 = mybir.dt.float32
BF16 = mybir.dt.bfloat16
FP8 = mybir.dt.float8e4
I16 = mybir.dt.int16
AF = mybir.ActivationFunctionType

GMAX = 8       # max chunks (of 128 idxs) per dma_gather call (ring limit 1024)
NQ = 4         # SWDGE queues
GRP = 8        # dst blocks per gather/compute group


class Cfg:
    def __init__(self, n_nodes=50000, n_cores=8, d=128, n_graphs=64):
        self.n_nodes = n_nodes
        self.n_cores = n_cores
        self.d = d
        self.n_graphs = n_graphs
        self.shard = n_nodes // n_cores          # 6250
        self.n_blk = (self.shard + 127) // 128   # 49
        self.shard_p = self.n_blk * 128          # 6272
        # node segments (per-core row ranges); each segment's global table
        # stays < 32768 rows (int16 gather indices), and the last segment is
        # small so its boundary AllGather is cheap
        self.seg_base = [0, 3072, 5760]
        self.seg_end = [3072, 5760, 6272]
        self.seg_rows = [e - b for b, e in zip(self.seg_base, self.seg_end)]
        self.n_seg = len(self.seg_base)
        self.seg_last_blk = [e // 128 - 1 for e in self.seg_end]
        self.n_grp = (self.n_blk + GRP - 1) // GRP


def bucket_order(cfg):
    """Stream order of (block, segment) buckets: (group, seg, block)."""
    order = []
    for g in range(cfg.n_grp):
        bs = range(g * GRP, min(cfg.n_blk, (g + 1) * GRP))
        for h in range(cfg.n_seg):
            for b in bs:
                order.append((b, h))
    return order


def preprocess(cfg, edge_index, edge_weight, x, batch):
    src = np.asarray(edge_index)[0].astype(np.int64)
    dst = np.asarray(edge_index)[1].astype(np.int64)
    ew = np.asarray(edge_weight).astype(np.float32)
    n, C, S = cfg.n_nodes, cfg.n_cores, cfg.shard
    ne = src.shape[0]

    deg = np.bincount(dst, weights=ew, minlength=n) + 1.0
    dinv = (1.0 / np.sqrt(deg)).astype(np.float32)     # [n]
    g0 = (np.asarray(x, np.float32) * dinv[:, None])   # [n, d] fp32

    # destination decomposition
    core = dst // S
    l = dst - core * S
    b = l // 128
    slot = l - b * 128
    # source -> (segment, row) in the segment tables
    sc = src // S
    r = src - sc * S
    half = np.digitize(r, cfg.seg_base[1:])        # segment id 0..n_seg-1
    sbase = np.array(cfg.seg_base)[half]
    srows = np.array(cfg.seg_rows)[half]
    row = sc * srows + (r - sbase)

    order = bucket_order(cfg)
    ns = cfg.n_seg
    bpos = np.zeros(cfg.n_blk * ns, np.int64)
    for i, (bb, hh) in enumerate(order):
        bpos[bb * ns + hh] = i
    skey = bpos[b * ns + half]                     # bucket stream index
    key = core * len(order) + skey

    # Dedupe (src -> dst-block): duplicate edges share one gathered row;
    # their one-hot row then carries multiple nonzeros (one per dst slot).
    osort = np.lexsort((row, key))
    keyo, rowo = key[osort], row[osort]
    lead = np.ones(ne, bool)
    lead[1:] = (keyo[1:] != keyo[:-1]) | (rowo[1:] != rowo[:-1])
    group = np.cumsum(lead) - 1                    # per sorted edge
    nl = int(lead.sum())
    lkey = keyo[lead]                              # bucket key per group

    cnt = np.bincount(lkey, minlength=C * len(order))
    cnt2 = cnt.reshape(C, len(order))
    seg = ((cnt2.max(axis=0) + 127) // 128) * 128   # [n_buckets] stream order
    seg_off = np.concatenate([[0], np.cumsum(seg)])
    ep = int(seg_off[-1])

    starts = np.concatenate([[0], np.cumsum(cnt)])[:-1]
    rank = np.arange(nl) - starts[lkey]
    gpos = seg_off[lkey % len(order)] + rank       # slot per unique group
    pos = gpos[group]                              # per sorted edge
    core_s = keyo // len(order)

    idx16 = np.zeros((C, ep), np.int16)
    idx16[core_s[lead], gpos] = rowo[lead].astype(np.int16)
    nchunk = ep // 128
    oh = np.zeros((C, 128, nchunk * 128), np.float32)
    np.add.at(oh, (core_s, pos % 128, (pos // 128) * 128 + slot[osort]),
              ew[osort])
    oh = oh.astype(NP_FP8)

    idxw = idx16.reshape(C, ep // 16, 16).transpose(0, 2, 1)
    idxw = np.tile(idxw, (1, 8, 1)).copy()          # [C, 128, ep//16]

    # per-core aux arrays: gown0[c, l%128, (l//128)*128 + f] = g0[c*S+l, f]
    batch = np.asarray(batch).astype(np.int64)
    lr = np.arange(S)
    li = np.tile(lr, C)
    ci = np.repeat(np.arange(C), S)
    gown0 = np.zeros((C, 128, cfg.n_blk * 128), np.float32)
    gown0[ci[:, None], (li % 128)[:, None],
          ((li // 128) * 128)[:, None] + np.arange(cfg.d)[None, :]] = g0[
        ci * S + li]
    dinvb = np.zeros((C, 128, cfg.n_blk), np.float32)
    dinvb[ci, li % 128, li // 128] = dinv[ci * S + li]
    poolm = np.zeros((C, 128, cfg.n_blk * cfg.n_graphs), np.float32)
    poolm[ci, li % 128, (li // 128) * cfg.n_graphs + batch[ci * S + li]] = 1.0

    # layer-0 segment tables (padded local rows are zero)
    gfs0 = []
    for s in range(cfg.n_seg):
        rows = cfg.seg_rows[s]
        t = np.zeros((C * rows, cfg.d), np.float32)
        rs = lr[(lr >= cfg.seg_base[s]) & (lr < cfg.seg_end[s])]
        for c in range(C):
            t[c * rows + (rs - cfg.seg_base[s])] = g0[c * S + rs]
        gfs0.append(t.astype(NP_BF16))

    counts = np.bincount(batch, minlength=cfg.n_graphs).astype(np.float32)
    return dict(seg=seg, seg_off=seg_off, ep=ep, nchunk=nchunk,
                idxw=idxw, oh=oh,
                gown0=gown0.astype(NP_BF16), dinvb=dinvb,
                poolm=poolm.astype(NP_BF16), gfs0=gfs0,
                counts=counts)


def build_program(cfg, seg, seg_off, ep):
    """SPMD Bass/Tile program; trip counts depend only on seg (shared)."""
    d, g64, n_blk = cfg.d, cfg.n_graphs, cfg.n_blk
    order = bucket_order(cfg)
    nchunk = ep // 128
    # chunk ranges per bucket (stream order)
    boff = {order[i]: int(seg_off[i]) // 128 for i in range(len(order))}
    bcnt = {order[i]: int(seg[i]) // 128 for i in range(len(order))}

    nc = bacc.Bacc("TRN2", target_bir_lowering=False, debug=False,
                   enable_asserts=False, num_devices=cfg.n_cores,
                   num_swdge_queues=NQ)

    gfs_in = [nc.dram_tensor(f"gfs0_{s}",
                             [cfg.n_cores * cfg.seg_rows[s], d], BF16,
                             kind="ExternalInput") for s in range(cfg.n_seg)]
    gown_in = nc.dram_tensor("gown0", [128, n_blk * 128], BF16,
                             kind="ExternalInput")
    idx_in = nc.dram_tensor("idxw", [128, ep // 16], I16, kind="ExternalInput")
    oh_in = nc.dram_tensor("oh", [128, nchunk * 128], FP8,
                           kind="ExternalInput")
    poolm_in = nc.dram_tensor("poolm", [128, n_blk * g64], BF16,
                              kind="ExternalInput")
    dinv_in = nc.dram_tensor("dinvb", [128, n_blk], F32, kind="ExternalInput")
    w_in = nc.dram_tensor("wmats", [3, d, d], BF16, kind="ExternalInput")
    b_in = nc.dram_tensor("biasb", [3, 128, d], F32, kind="ExternalInput")
    id_in = nc.dram_tensor("ident", [128, 128], BF16, kind="ExternalInput")
    out_t = nc.dram_tensor("pool_out", [g64, d], F32, kind="ExternalOutput")

    g_loc = [[nc.dram_tensor(f"g_loc{k}_{s}", [cfg.seg_rows[s], d], BF16,
                             kind="Internal") for s in range(cfg.n_seg)]
             for k in (1, 2)]
    gfs = [[nc.dram_tensor(f"gfs{k}_{s}",
                           [cfg.n_cores * cfg.seg_rows[s], d], BF16,
                           kind="Internal", addr_space="Shared")
            for s in range(cfg.n_seg)] for k in (1, 2)]
    rg = [list(range(cfg.n_cores))]
    # Collective schedule.  Segment s's AllGather must be EMITTED after the
    # block loop that writes its last g_loc rows; beyond that it is deferred
    # (emission point = after the NEXT group's seg-0 gather calls) so the
    # in-order Pool queue overlaps the semaphore wait with useful desc-gen.
    # Segments finishing in the final group are emitted inside the next
    # layer's first group (their data is only needed by its seg>=1 calls).
    ag_sched = {}   # group index -> segments, emitted before group's gathers
    ag_tail = []    # deferred into the next layer's group 0
    for s in range(cfg.n_seg):
        last_grp = cfg.seg_last_blk[s] // GRP
        if last_grp >= cfg.n_grp - 1:
            ag_tail.append(s)
        else:
            ag_sched.setdefault(min(last_grp + 2, cfg.n_grp - 1), []).append(s)

    with tile.TileContext(nc) as tc:
        with tc.tile_pool(name="const", bufs=1) as cp:
            idx_sb = cp.tile([128, ep // 16], I16, tag="idx")
            # split the idx load so the first group's gathers start sooner
            c_sp = max(8, (ep // 16) // 8)
            nc.sync.dma_start(idx_sb[:, :c_sp], idx_in.ap()[:, :c_sp])
            nc.sync.dma_start(idx_sb[:, c_sp:], idx_in.ap()[:, c_sp:])
            poolm_sb = cp.tile([128, n_blk * g64], BF16, tag="poolm")
            nc.sync.dma_start(poolm_sb[:, :], poolm_in.ap())
            dinv_sb = cp.tile([128, n_blk], F32, tag="dinv")
            nc.sync.dma_start(dinv_sb[:, :], dinv_in.ap())
            i128_sb = cp.tile([128, 128], BF16, tag="i128")
            nc.sync.dma_start(i128_sb[:, :], id_in.ap())
            w_sb, b_sb = [], []
            for k in range(3):
                wt = cp.tile([d, d], BF16, tag=f"w{k}", name=f"w{k}")
                nc.sync.dma_start(wt[:, :], w_in.ap()[k, :, :])
                w_sb.append(wt)
                bt = cp.tile([128, d], F32, tag=f"b{k}", name=f"b{k}")
                nc.sync.dma_start(bt[:, :], b_in.ap()[k, :, :])
                b_sb.append(bt)
            gpp = [[cp.tile([128, 128], BF16, tag=f"gown{i}_{b}",
                            name=f"gown{i}_{b}") for b in range(n_blk)]
                   for i in (0, 1)]
            for b in range(n_blk):
                nc.sync.dma_start(gpp[0][b][:, :],
                                  gown_in.ap()[:, b * 128:(b + 1) * 128])

            with tc.tile_pool(name="stage", bufs=30) as stp, \
                 tc.tile_pool(name="ohp", bufs=3) as ohp, \
                 tc.tile_pool(name="aggp", bufs=5, space="PSUM") as psA, \
                 tc.tile_pool(name="outp", bufs=2, space="PSUM") as psB, \
                 tc.tile_pool(name="poolp", bufs=1, space="PSUM") as psC, \
                 tc.tile_pool(name="work", bufs=6) as wp:
                qrr = [0]
                pp = None
                pending_ags = []
                for k in range(3):
                    tbl = [gfs_in[s].ap() if k == 0 else gfs[k - 1][s].ap()
                           for s in range(cfg.n_seg)]
                    gcur = gpp[k % 2]
                    gnxt = gpp[(k + 1) % 2]
                    for g in range(cfg.n_grp):
                        bs = range(g * GRP, min(n_blk, (g + 1) * GRP))
                        if k < 2:
                            for s in ag_sched.get(g, []):
                                nc.gpsimd.collective_compute(
                                    "AllGather", AluOpType.bypass,
                                    replica_groups=rg,
                                    ins=[g_loc[k][s].ap()],
                                    outs=[gfs[k][s].ap()])
                        chunkmap = {}
                        for h in range(cfg.n_seg):
                            if h == 1 and g == 0 and pending_ags:
                                # the previous layer's tail collective: its
                                # data is first needed by this group's seg>=1
                                # calls; emitting it here overlaps its wait
                                # with the seg-0 desc-gen
                                for kk, s in pending_ags:
                                    nc.gpsimd.collective_compute(
                                        "AllGather", AluOpType.bypass,
                                        replica_groups=rg,
                                        ins=[g_loc[kk][s].ap()],
                                        outs=[gfs[kk][s].ap()])
                                pending_ags = []
                            c0 = boff[(bs[0], h)]
                            c1 = boff[(bs[-1], h)] + bcnt[(bs[-1], h)]
                            base = tbl[h]
                            for cc in range(c0, c1, GMAX):
                                ncall = min(GMAX, c1 - cc)
                                st = stp.tile([128, ncall, 128], BF16,
                                              tag="st",
                                              name=f"st{k}_{g}_{h}_{cc}")
                                nc.gpsimd.dma_gather(
                                    st[:, :, :], base,
                                    idx_sb[:, cc * 8:(cc + ncall) * 8],
                                    ncall * 128, ncall * 128, d,
                                    queue_num=qrr[0] % NQ)
                                qrr[0] += 1
                                for j in range(ncall):
                                    chunkmap[cc + j] = (st, j)
                        for b in bs:
                            nch = sum(bcnt[(b, h)] for h in range(cfg.n_seg))
                            ohb = ohp.tile([128, nch * 128], FP8, tag="ohb",
                                           name=f"oh{k}_{b}")
                            o = 0
                            for h in range(cfg.n_seg):
                                nh = bcnt[(b, h)]
                                if nh == 0:
                                    continue
                                nc.sync.dma_start(
                                    ohb[:, o * 128:(o + nh) * 128],
                                    oh_in.ap()[:, boff[(b, h)] * 128:
                                               (boff[(b, h)] + nh) * 128])
                                o += nh
                            pagg = psA.tile([128, 128], F32, tag="agg",
                                            name=f"agg{k}_{b}")
                            j = 0
                            for h in range(cfg.n_seg):
                                for i in range(bcnt[(b, h)]):
                                    st, jj = chunkmap[boff[(b, h)] + i]
                                    nc.tensor.matmul(
                                        pagg[:, :], st[:, jj, :],
                                        ohb[:, j * 128:(j + 1) * 128],
                                        start=(j == 0), stop=False)
                                    j += 1
                            nc.tensor.matmul(
                                pagg[:, :], gcur[b][:, :],
                                i128_sb[:, :], start=(j == 0), stop=True)
                            aggT = wp.tile([128, 128], BF16, tag="aggT",
                                           name=f"aggT{k}_{b}")
                            nc.scalar.copy(aggT[:, :], pagg[:, :])
                            pout = psB.tile([128, d], F32, tag="out",
                                            name=f"out{k}_{b}")
                            nc.tensor.matmul(pout[:, :], aggT[:, :],
                                             w_sb[k][:, :], start=True,
                                             stop=True)
                            t2 = wp.tile([128, d], BF16, tag="t2",
                                         name=f"t2{k}_{b}")
                            nc.vector.scalar_tensor_tensor(
                                t2[:, :], pout[:, :], dinv_sb[:, b:b + 1],
                                b_sb[k][:, :], AluOpType.mult, AluOpType.add)
                            if k < 2:
                                # g_next = dinv*relu(t2) = relu(dinv*t2)
                                gt = gnxt[b]
                                nc.scalar.activation(
                                    gt[:, :], t2[:, :], AF.Relu,
                                    scale=dinv_sb[:, b:b + 1])
                                s = next(i for i in range(cfg.n_seg)
                                         if b * 128 < cfg.seg_end[i])
                                r0 = b * 128 - cfg.seg_base[s]
                                nc.sync.dma_start(
                                    g_loc[k][s].ap()[r0:r0 + 128, :],
                                    gt[:, :])
                            else:
                                if pp is None:
                                    pp = psC.tile([g64, d], F32, tag="pp")
                                nc.tensor.matmul(
                                    pp[:, :],
                                    poolm_sb[:, b * g64:(b + 1) * g64],
                                    t2[:, :], start=(b == 0),
                                    stop=(b == n_blk - 1))
                    if k < 2:
                        pending_ags += [(k, s) for s in ag_tail]
                ppsb = cp.tile([g64, d], F32, tag="ppsb")
                nc.scalar.copy(ppsb[:, :], pp[:, :])
                nc.sync.dma_start(out_t.ap(), ppsb[:, :])

    nc.compile()
    return nc


def make_in_maps(cfg, prep, ws, bs):
    wmats = np.stack([np.asarray(w, np.float32) for w in ws]).astype(NP_BF16)
    biasb = np.stack([np.broadcast_to(np.asarray(b, np.float32),
                                      (128, cfg.d)) for b in bs]).copy()
    ident = np.eye(128, dtype=np.float32).astype(NP_BF16)
    in_maps = []
    for c in range(cfg.n_cores):
        in_maps.append({
            **{f"gfs0_{s}": prep["gfs0"][s] for s in range(cfg.n_seg)},
            "gown0": prep["gown0"][c], "idxw": prep["idxw"][c],
            "oh": prep["oh"][c], "poolm": prep["poolm"][c],
            "dinvb": prep["dinvb"][c], "wmats": wmats, "biasb": biasb,
            "ident": ident,
        })
    return in_maps


_PROGRAM_CACHE = {}


def run(cfg, x, edge_index, edge_weight, batch, ws, bs, trace=False, trunc=""):
    prep = preprocess(cfg, edge_index, edge_weight, x, batch)
    key = (cfg.n_nodes, cfg.n_cores, prep["ep"], tuple(prep["seg"]))
    nc = _PROGRAM_CACHE.get(key)
    if nc is None:
        nc = build_program(cfg, prep["seg"], prep["seg_off"], prep["ep"])
        _PROGRAM_CACHE[key] = nc
    in_maps = make_in_maps(cfg, prep, ws, bs)
    res = bass_utils.run_bass_kernel_spmd(
        nc, in_maps, core_ids=list(range(cfg.n_cores)), trace=trace)
    partial = np.zeros((cfg.n_graphs, cfg.d), np.float64)
    for c in range(cfg.n_cores):
        partial += res.results[c]["pool_out"].astype(np.float64)
    out = (partial / np.maximum(prep["counts"], 1.0)[:, None]).astype(
        np.float32)
    return out, res


def kernel(x, edge_index, edge_weight, batch, W0, b0, W1, b1, W2, b2):
    cfg = Cfg()
    trace = bool(int(os.environ.get("GCN_TRACE", "0")))
    out, _ = run(cfg, x, edge_index, edge_weight, batch,
                 [W0, W1, W2], [b0, b1, b2], trace=trace)
    return out


# revision 47
# speedup vs baseline: 1.1479x; 1.0426x over previous
"""Trainium2 Bass kernel for a 3-layer GCN encoder (PyG GCNConv x3 + global mean pool).

Strategy (8 NeuronCores, v2):
  - Nodes sharded contiguously (6250/core, padded to 6272 = 49 blocks of 128);
    edges partitioned by destination, bucketed per (dst-block, src-segment).
  - Per layer k:  out = A_hat @ (g @ W) + b  with g = dinv * h, reassociated as
    (A_hat @ g) @ W.  Per dst block:
        agg[f, slot] = sum_e g[src_e, f] * OH[e, slot]      (PE, bf16)
                     + g_own[slot -> f] @ I                 (self-loops, PE)
        h' = relu(dinv * (agg @ W) + b)
    The per-edge one-hot OH (ew at [e, slot]) is HOST-precomputed in bf16 and
    streamed from HBM; no on-device one-hot construction at all.
  - Per-edge rows gathered from HBM in bf16 (256 B descriptors) via SWDGE
    dma_gather, 1024 idxs/call, 4 queues, deep staging (measured floor
    ~2.8 ns/descriptor on the Pool engine -- the kernel's critical resource).
  - deg/dinv and g0 = dinv*x are host-side preprocessing (edge metadata and an
    elementwise input scale); all matmuls/aggregation run on device.
  - The gathered-feature table is AllGathered between layers in bf16, split in
    two segments (A: blocks 0-23, B: 24-48) so segment A's collective overlaps
    with the tail half of the layer's compute.  The segment split also keeps
    gather indices within int16 (rows < 32768 per segment table).
  - Final global mean pool: per-core one-hot matmul into [64, 128]; host sums
    the 8 partials and divides by counts.
"""

import os
import sys

import numpy as np
import ml_dtypes

NP_BF16 = ml_dtypes.bfloat16
NP_FP8 = ml_dtypes.float8_e4m3fn

for _p in ("/opt/trn_rl_repo",):
    if _p not in sys.path and os.path.isdir(_p):
        sys.path.insert(0, _p)

import concourse.bass as bass
import concourse.bacc as bacc
import concourse.tile as tile
import concourse.mybir as mybir
from concourse import bass_utils
from concourse.alu_op_type import AluOpType

F32 = mybir.dt.float32
BF16 = mybir.dt.bfloat16
FP8 = mybir.dt.float8e4
I16 = mybir.dt.int16
AF = mybir.ActivationFunctionType

GMAX = 8       # max chunks (of 128 idxs) per dma_gather call (ring limit 1024)
NQ = 4         # SWDGE queues
GRP = 8        # dst blocks per gather/compute group


class Cfg:
    def __init__(self, n_nodes=50000, n_cores=8, d=128, n_graphs=64):
        self.n_nodes = n_nodes
        self.n_cores = n_cores
        self.d = d
        self.n_graphs = n_graphs
        self.shard = n_nodes // n_cores          # 6250
        self.n_blk = (self.shard + 127) // 128   # 49
        self.shard_p = self.n_blk * 128          # 6272
        # node segments (per-core row ranges); each segment's global table
        # stays < 32768 rows (int16 gather indices), and the last segment is
        # small so its boundary AllGather is cheap
        self.seg_base = [0, 3072, 5760]
        self.seg_end = [3072, 5760, 6272]
        self.seg_rows = [e - b for b, e in zip(self.seg_base, self.seg_end)]
        self.n_seg = len(self.seg_base)
        self.seg_last_blk = [e // 128 - 1 for e in self.seg_end]
        self.n_grp = (self.n_blk + GRP - 1) // GRP


def bucket_order(cfg):
    """Stream order of (block, segment) buckets: (group, seg, block)."""
    order = []
    for g in range(cfg.n_grp):
        bs = range(g * GRP, min(cfg.n_blk, (g + 1) * GRP))
        for h in range(cfg.n_seg):
            for b in bs:
                order.append((b, h))
    return order


def preprocess(cfg, edge_index, edge_weight, x, batch):
    src = np.asarray(edge_index)[0].astype(np.int64)
    dst = np.asarray(edge_index)[1].astype(np.int64)
    ew = np.asarray(edge_weight).astype(np.float32)
    n, C, S = cfg.n_nodes, cfg.n_cores, cfg.shard
    ne = src.shape[0]

    deg = np.bincount(dst, weights=ew, minlength=n) + 1.0
    dinv = (1.0 / np.sqrt(deg)).astype(np.float32)     # [n]
    g0 = (np.asarray(x, np.float32) * dinv[:, None])   # [n, d] fp32

    # destination decomposition
    core = dst // S
    l = dst - core * S
    b = l // 128
    slot = l - b * 128
    # source -> (segment, row) in the segment tables
    sc = src // S
    r = src - sc * S
    half = np.digitize(r, cfg.seg_base[1:])        # segment id 0..n_seg-1
    sbase = np.array(cfg.seg_base)[half]
    srows = np.array(cfg.seg_rows)[half]
    row = sc * srows + (r - sbase)

    order = bucket_order(cfg)
    ns = cfg.n_seg
    bpos = np.zeros(cfg.n_blk * ns, np.int64)
    for i, (bb, hh) in enumerate(order):
        bpos[bb * ns + hh] = i
    skey = bpos[b * ns + half]                     # bucket stream index
    key = core * len(order) + skey

    # Dedupe (src -> dst-block): duplicate edges share one gathered row;
    # their one-hot row then carries multiple nonzeros (one per dst slot).
    osort = np.lexsort((row, key))
    keyo, rowo = key[osort], row[osort]
    lead = np.ones(ne, bool)
    lead[1:] = (keyo[1:] != keyo[:-1]) | (rowo[1:] != rowo[:-1])
    group = np.cumsum(lead) - 1                    # per sorted edge
    nl = int(lead.sum())
    lkey = keyo[lead]                              # bucket key per group

    cnt = np.bincount(lkey, minlength=C * len(order))
    cnt2 = cnt.reshape(C, len(order))
    seg = ((cnt2.max(axis=0) + 127) // 128) * 128   # [n_buckets] stream order
    seg_off = np.concatenate([[0], np.cumsum(seg)])
    ep = int(seg_off[-1])

    starts = np.concatenate([[0], np.cumsum(cnt)])[:-1]
    rank = np.arange(nl) - starts[lkey]
    gpos = seg_off[lkey % len(order)] + rank       # slot per unique group
    pos = gpos[group]                              # per sorted edge
    core_s = keyo // len(order)

    idx16 = np.zeros((C, ep), np.int16)
    idx16[core_s[lead], gpos] = rowo[lead].astype(np.int16)
    nchunk = ep // 128
    oh = np.zeros((C, 128, nchunk * 128), np.float32)
    np.add.at(oh, (core_s, pos % 128, (pos // 128) * 128 + slot[osort]),
              ew[osort])
    oh = oh.astype(NP_FP8)

    idxw = idx16.reshape(C, ep // 16, 16).transpose(0, 2, 1)
    idxw = np.tile(idxw, (1, 8, 1)).copy()          # [C, 128, ep//16]

    # per-core aux arrays: gown0[c, l%128, (l//128)*128 + f] = g0[c*S+l, f]
    batch = np.asarray(batch).astype(np.int64)
    lr = np.arange(S)
    li = np.tile(lr, C)
    ci = np.repeat(np.arange(C), S)
    gown0 = np.zeros((C, 128, cfg.n_blk * 128), np.float32)
    gown0[ci[:, None], (li % 128)[:, None],
          ((li // 128) * 128)[:, None] + np.arange(cfg.d)[None, :]] = g0[
        ci * S + li]
    dinvb = np.zeros((C, 128, cfg.n_blk), np.float32)
    dinvb[ci, li % 128, li // 128] = dinv[ci * S + li]
    poolm = np.zeros((C, 128, cfg.n_blk * cfg.n_graphs), np.float32)
    poolm[ci, li % 128, (li // 128) * cfg.n_graphs + batch[ci * S + li]] = 1.0

    # layer-0 segment tables (padded local rows are zero)
    gfs0 = []
    for s in range(cfg.n_seg):
        rows = cfg.seg_rows[s]
        t = np.zeros((C * rows, cfg.d), np.float32)
        rs = lr[(lr >= cfg.seg_base[s]) & (lr < cfg.seg_end[s])]
        for c in range(C):
            t[c * rows + (rs - cfg.seg_base[s])] = g0[c * S + rs]
        gfs0.append(t.astype(NP_BF16))

    counts = np.bincount(batch, minlength=cfg.n_graphs).astype(np.float32)
    return dict(seg=seg, seg_off=seg_off, ep=ep, nchunk=nchunk,
                idxw=idxw, oh=oh,
                gown0=gown0.astype(NP_BF16), dinvb=dinvb,
                poolm=poolm.astype(NP_BF16), gfs0=gfs0,
                counts=counts)


def build_program(cfg, seg, seg_off, ep):
    """SPMD Bass/Tile program; trip counts depend only on seg (shared)."""
    d, g64, n_blk = cfg.d, cfg.n_graphs, cfg.n_blk
    order = bucket_order(cfg)
    nchunk = ep // 128
    # chunk ranges per bucket (stream order)
    boff = {order[i]: int(seg_off[i]) // 128 for i in range(len(order))}
    bcnt = {order[i]: int(seg[i]) // 128 for i in range(len(order))}

    nc = bacc.Bacc("TRN2", target_bir_lowering=False, debug=False,
                   enable_asserts=False, num_devices=cfg.n_cores,
                   num_swdge_queues=NQ)

    gfs_in = [nc.dram_tensor(f"gfs0_{s}",
                             [cfg.n_cores * cfg.seg_rows[s], d], BF16,
                             kind="ExternalInput") for s in range(cfg.n_seg)]
    gown_in = nc.dram_tensor("gown0", [128, n_blk * 128], BF16,
                             kind="ExternalInput")
    idx_in = nc.dram_tensor("idxw", [128, ep // 16], I16, kind="ExternalInput")
    oh_in = nc.dram_tensor("oh", [128, nchunk * 128], FP8,
                           kind="ExternalInput")
    poolm_in = nc.dram_tensor("poolm", [128, n_blk * g64], BF16,
                              kind="ExternalInput")
    dinv_in = nc.dram_tensor("dinvb", [128, n_blk], F32, kind="ExternalInput")
    w_in = nc.dram_tensor("wmats", [3, d, d], BF16, kind="ExternalInput")
    b_in = nc.dram_tensor("biasb", [3, 128, d], F32, kind="ExternalInput")
    id_in = nc.dram_tensor("ident", [128, 128], BF16, kind="ExternalInput")
    out_t = nc.dram_tensor("pool_out", [g64, d], F32, kind="ExternalOutput")

    g_loc = [[nc.dram_tensor(f"g_loc{k}_{s}", [cfg.seg_rows[s], d], BF16,
                             kind="Internal") for s in range(cfg.n_seg)]
             for k in (1, 2)]
    gfs = [[nc.dram_tensor(f"gfs{k}_{s}",
                           [cfg.n_cores * cfg.seg_rows[s], d], BF16,
                           kind="Internal", addr_space="Shared")
            for s in range(cfg.n_seg)] for k in (1, 2)]
    rg = [list(range(cfg.n_cores))]
    # Collective schedule.  Segment s's AllGather must be EMITTED after the
    # block loop that writes its last g_loc rows; beyond that it is deferred
    # (emission point = after the NEXT group's seg-0 gather calls) so the
    # in-order Pool queue overlaps the semaphore wait with useful desc-gen.
    # Segments finishing in the final group are emitted inside the next
    # layer's first group (their data is only needed by its seg>=1 calls).
    ag_sched = {}   # group index -> segments, emitted before group's gathers
    ag_tail = []    # deferred into the next layer's group 0
    for s in range(cfg.n_seg):
        last_grp = cfg.seg_last_blk[s] // GRP
        if last_grp >= cfg.n_grp - 1:
            ag_tail.append(s)
        else:
            ag_sched.setdefault(min(last_grp + 2, cfg.n_grp - 1), []).append(s)

    with tile.TileContext(nc) as tc:
        with tc.tile_pool(name="const", bufs=1) as cp:
            idx_sb = cp.tile([128, ep // 16], I16, tag="idx")
            # split the idx load so the first group's gathers start sooner
            c_sp = max(8, (ep // 16) // 8)
            nc.sync.dma_start(idx_sb[:, :c_sp], idx_in.ap()[:, :c_sp])
            nc.sync.dma_start(idx_sb[:, c_sp:], idx_in.ap()[:, c_sp:])
            poolm_sb = cp.tile([128, n_blk * g64], BF16, tag="poolm")
            nc.sync.dma_start(poolm_sb[:, :], poolm_in.ap())
            dinv_sb = cp.tile([128, n_blk], F32, tag="dinv")
            nc.sync.dma_start(dinv_sb[:, :], dinv_in.ap())
            i128_sb = cp.tile([128, 128], BF16, tag="i128")
            nc.sync.dma_start(i128_sb[:, :], id_in.ap())
            w_sb, b_sb = [], []
            for k in range(3):
                wt = cp.tile([d, d], BF16, tag=f"w{k}", name=f"w{k}")
                nc.sync.dma_start(wt[:, :], w_in.ap()[k, :, :])
                w_sb.append(wt)
                bt = cp.tile([128, d], F32, tag=f"b{k}", name=f"b{k}")
                nc.sync.dma_start(bt[:, :], b_in.ap()[k, :, :])
                b_sb.append(bt)
            gpp = [[cp.tile([128, 128], BF16, tag=f"gown{i}_{b}",
                            name=f"gown{i}_{b}") for b in range(n_blk)]
                   for i in (0, 1)]
            for b in range(n_blk):
                nc.sync.dma_start(gpp[0][b][:, :],
                                  gown_in.ap()[:, b * 128:(b + 1) * 128])

            with tc.tile_pool(name="stage", bufs=30) as stp, \
                 tc.tile_pool(name="ohp", bufs=3) as ohp, \
                 tc.tile_pool(name="aggp", bufs=5, space="PSUM") as psA, \
                 tc.tile_pool(name="outp", bufs=2, space="PSUM") as psB, \
                 tc.tile_pool(name="poolp", bufs=1, space="PSUM") as psC, \
                 tc.tile_pool(name="work", bufs=6) as wp:
                qrr = [0]
                pp = None
                pending_ags = []
                for k in range(3):
                    tbl = [gfs_in[s].ap() if k == 0 else gfs[k - 1][s].ap()
                           for s in range(cfg.n_seg)]
                    gcur = gpp[k % 2]
                    gnxt = gpp[(k + 1) % 2]
                    for g in range(cfg.n_grp):
                        bs = range(g * GRP, min(n_blk, (g + 1) * GRP))
                        if k < 2:
                            for s in ag_sched.get(g, []):
                                nc.gpsimd.collective_compute(
                                    "AllGather", AluOpType.bypass,
                                    replica_groups=rg,
                                    ins=[g_loc[k][s].ap()],
                                    outs=[gfs[k][s].ap()])
                        chunkmap = {}
                        for h in range(cfg.n_seg):
                            c0 = boff[(bs[0], h)]
                            c1 = boff[(bs[-1], h)] + bcnt[(bs[-1], h)]
                            base = tbl[h]
                            for cc in range(c0, c1, GMAX):
                                ncall = min(GMAX, c1 - cc)
                                st = stp.tile([128, ncall, 128], BF16,
                                              tag="st",
                                              name=f"st{k}_{g}_{h}_{cc}")
                                nc.gpsimd.dma_gather(
                                    st[:, :, :], base,
                                    idx_sb[:, cc * 8:(cc + ncall) * 8],
                                    ncall * 128, ncall * 128, d,
                                    queue_num=qrr[0] % NQ)
                                qrr[0] += 1
                                for j in range(ncall):
                                    chunkmap[cc + j] = (st, j)
                        for b in bs:
                            nch = sum(bcnt[(b, h)] for h in range(cfg.n_seg))
                            ohb = ohp.tile([128, nch * 128], FP8, tag="ohb",
                                           name=f"oh{k}_{b}")
                            o = 0
                            for h in range(cfg.n_seg):
                                nh = bcnt[(b, h)]
                                if nh == 0:
                                    continue
                                nc.sync.dma_start(
                                    ohb[:, o * 128:(o + nh) * 128],
                                    oh_in.ap()[:, boff[(b, h)] * 128:
                                               (boff[(b, h)] + nh) * 128])
                                o += nh
                            pagg = psA.tile([128, 128], F32, tag="agg",
                                            name=f"agg{k}_{b}")
                            j = 0
                            for h in range(cfg.n_seg):
                                for i in range(bcnt[(b, h)]):
                                    st, jj = chunkmap[boff[(b, h)] + i]
                                    nc.tensor.matmul(
                                        pagg[:, :], st[:, jj, :],
                                        ohb[:, j * 128:(j + 1) * 128],
                                        start=(j == 0), stop=False)
                                    j += 1
                            nc.tensor.matmul(
                                pagg[:, :], gcur[b][:, :],
                                i128_sb[:, :], start=(j == 0), stop=True)
                            aggT = wp.tile([128, 128], BF16, tag="aggT",
                                           name=f"aggT{k}_{b}")
                            nc.scalar.copy(aggT[:, :], pagg[:, :])
                            pout = psB.tile([128, d], F32, tag="out",
                                            name=f"out{k}_{b}")
                            nc.tensor.matmul(pout[:, :], aggT[:, :],
                                             w_sb[k][:, :], start=True,
                                             stop=True)
                            t2 = wp.tile([128, d], BF16, tag="t2",
                                         name=f"t2{k}_{b}")
                            nc.vector.scalar_tensor_tensor(
                                t2[:, :], pout[:, :], dinv_sb[:, b:b + 1],
                                b_sb[k][:, :], AluOpType.mult, AluOpType.add)
                            if k < 2:
                                # g_next = dinv*relu(t2) = relu(dinv*t2)
                                gt = gnxt[b]
                                nc.scalar.activation(
                                    gt[:, :], t2[:, :], AF.Relu,
                                    scale=dinv_sb[:, b:b + 1])
                                s = next(i for i in range(cfg.n_seg)
                                         if b * 128 < cfg.seg_end[i])
                                r0 = b * 128 - cfg.seg_base[s]
                                nc.sync.dma_start(
                                    g_loc[k][s].ap()[r0:r0 + 128, :],
                                    gt[:, :])
                            else:
                                if pp is None:
                                    pp = psC.tile([g64, d], F32, tag="pp")
                                nc.tensor.matmul(
                                    pp[:, :],
                                    poolm_sb[:, b * g64:(b + 1) * g64],
                                    t2[:, :], start=(b == 0),
                                    stop=(b == n_blk - 1))
                    if k < 2:
                        for s in ag_tail:
                            nc.gpsimd.collective_compute(
                                "AllGather", AluOpType.bypass,
                                replica_groups=rg,
                                ins=[g_loc[k][s].ap()],
                                outs=[gfs[k][s].ap()])
                ppsb = cp.tile([g64, d], F32, tag="ppsb")
                nc.scalar.copy(ppsb[:, :], pp[:, :])
                nc.sync.dma_start(out_t.ap(), ppsb[:, :])

    nc.compile()
    return nc


def make_in_maps(cfg, prep, ws, bs):
    wmats = np.stack([np.asarray(w, np.float32) for w in ws]).astype(NP_BF16)
    biasb = np.stack([np.broadcast_to(np.asarray(b, np.float32),
                                      (128, cfg.d)) for b in bs]).copy()
    ident = np.eye(128, dtype=np.float32).astype(NP_BF16)
    in_maps = []
    for c in range(cfg.n_cores):
        in_maps.append({
            **{f"gfs0_{s}": prep["gfs0"][s] for s in range(cfg.n_seg)},
            "gown0": prep["gown0"][c], "idxw": prep["idxw"][c],
            "oh": prep["oh"][c], "poolm": prep["poolm"][c],
            "dinvb": prep["dinvb"][c], "wmats": wmats, "biasb": biasb,
            "ident": ident,
        })
    return in_maps


_PROGRAM_CACHE = {}


def run(cfg, x, edge_index, edge_weight, batch, ws, bs, trace=False, trunc=""):
    prep = preprocess(cfg, edge_index, edge_weight, x, batch)
    key = (cfg.n_nodes, cfg.n_cores, prep["ep"], tuple(prep["seg"]))
    nc = _PROGRAM_CACHE.get(key)
    if nc is None:
        nc = build_program(cfg, prep["seg"], prep["seg_off"], prep["ep"])
        _PROGRAM_CACHE[key] = nc
    in_maps = make_in_maps(cfg, prep, ws, bs)
    res = bass_utils.run_bass_kernel_spmd(
        nc, in_maps, core_ids=list(range(cfg.n_cores)), trace=trace)
    partial = np.zeros((cfg.n_graphs, cfg.d), np.float64)
    for c in range(cfg.n_cores):
        partial += res.results[c]["pool_out"].astype(np.float64)
    out = (partial / np.maximum(prep["counts"], 1.0)[:, None]).astype(
        np.float32)
    return out, res


def kernel(x, edge_index, edge_weight, batch, W0, b0, W1, b1, W2, b2):
    cfg = Cfg()
    trace = bool(int(os.environ.get("GCN_TRACE", "0")))
    out, _ = run(cfg, x, edge_index, edge_weight, batch,
                 [W0, W1, W2], [b0, b1, b2], trace=trace)
    return out


# revision 56
# speedup vs baseline: 1.3018x; 1.1341x over previous
"""Trainium2 Bass kernel for a 3-layer GCN encoder (PyG GCNConv x3 + global mean pool).

Strategy (8 NeuronCores, v2):
  - Nodes sharded contiguously (6250/core, padded to 6272 = 49 blocks of 128);
    edges partitioned by destination, bucketed per (dst-block, src-segment).
  - Per layer k:  out = A_hat @ (g @ W) + b  with g = dinv * h, reassociated as
    (A_hat @ g) @ W.  Per dst block:
        agg[f, slot] = sum_e g[src_e, f] * OH[e, slot]      (PE, bf16)
                     + g_own[slot -> f] @ I                 (self-loops, PE)
        h' = relu(dinv * (agg @ W) + b)
    The per-edge one-hot OH (ew at [e, slot]) is HOST-precomputed in bf16 and
    streamed from HBM; no on-device one-hot construction at all.
  - Per-edge rows gathered from HBM in bf16 (256 B descriptors) via SWDGE
    dma_gather, 1024 idxs/call, 4 queues, deep staging (measured floor
    ~2.8 ns/descriptor on the Pool engine -- the kernel's critical resource).
  - deg/dinv and g0 = dinv*x are host-side preprocessing (edge metadata and an
    elementwise input scale); all matmuls/aggregation run on device.
  - The gathered-feature table is AllGathered between layers in bf16, split in
    two segments (A: blocks 0-23, B: 24-48) so segment A's collective overlaps
    with the tail half of the layer's compute.  The segment split also keeps
    gather indices within int16 (rows < 32768 per segment table).
  - Final global mean pool: per-core one-hot matmul into [64, 128]; host sums
    the 8 partials and divides by counts.
"""

import os
import sys

import numpy as np
import ml_dtypes

NP_BF16 = ml_dtypes.bfloat16
NP_FP8 = ml_dtypes.float8_e4m3fn

for _p in ("/opt/trn_rl_repo",):
    if _p not in sys.path and os.path.isdir(_p):
        sys.path.insert(0, _p)

import concourse.bass as bass
import concourse.bacc as bacc
import concourse.tile as tile
import concourse.mybir as mybir
from concourse import bass_utils
from concourse.alu_op_type import AluOpType

F32 = mybir.dt.float32
BF16 = mybir.dt.bfloat16
FP8 = mybir.dt.float8e4
I16 = mybir.dt.int16
AF = mybir.ActivationFunctionType

GMAX = 8       # max chunks (of 128 idxs) per dma_gather call (ring limit 1024)
NQ = 4         # SWDGE queues
GRP = 8        # dst blocks per gather/compute group


class Cfg:
    def __init__(self, n_nodes=50000, n_cores=8, d=128, n_graphs=64):
        self.n_nodes = n_nodes
        self.n_cores = n_cores
        self.d = d
        self.n_graphs = n_graphs
        self.shard = n_nodes // n_cores          # 6250
        self.n_blk = (self.shard + 127) // 128   # 49
        self.shard_p = self.n_blk * 128          # 6272
        # node segments (per-core row ranges); each segment's global table
        # stays < 32768 rows (int16 gather indices), and the last segment is
        # small so its boundary AllGather is cheap
        self.seg_base = [0, 3072, 5760]
        self.seg_end = [3072, 5760, 6272]
        self.seg_rows = [e - b for b, e in zip(self.seg_base, self.seg_end)]
        self.n_seg = len(self.seg_base)
        self.seg_last_blk = [e // 128 - 1 for e in self.seg_end]
        self.n_grp = (self.n_blk + GRP - 1) // GRP


def bucket_order(cfg):
    """Stream order of (block, segment) buckets: (group, seg, block)."""
    order = []
    for g in range(cfg.n_grp):
        bs = range(g * GRP, min(cfg.n_blk, (g + 1) * GRP))
        for h in range(cfg.n_seg):
            for b in bs:
                order.append((b, h))
    return order


def preprocess(cfg, edge_index, edge_weight, x, batch):
    src = np.asarray(edge_index)[0].astype(np.int64)
    dst = np.asarray(edge_index)[1].astype(np.int64)
    ew = np.asarray(edge_weight).astype(np.float32)
    n, C, S = cfg.n_nodes, cfg.n_cores, cfg.shard
    ne = src.shape[0]

    deg = np.bincount(dst, weights=ew, minlength=n) + 1.0
    dinv = (1.0 / np.sqrt(deg)).astype(np.float32)     # [n]
    g0 = (np.asarray(x, np.float32) * dinv[:, None])   # [n, d] fp32

    # destination decomposition
    core = dst // S
    l = dst - core * S
    b = l // 128
    slot = l - b * 128
    # source -> (segment, row) in the segment tables
    sc = src // S
    r = src - sc * S
    half = np.digitize(r, cfg.seg_base[1:])        # segment id 0..n_seg-1
    sbase = np.array(cfg.seg_base)[half]
    srows = np.array(cfg.seg_rows)[half]
    row = sc * srows + (r - sbase)

    order = bucket_order(cfg)
    ns = cfg.n_seg
    bpos = np.zeros(cfg.n_blk * ns, np.int64)
    for i, (bb, hh) in enumerate(order):
        bpos[bb * ns + hh] = i
    skey = bpos[b * ns + half]                     # bucket stream index
    key = core * len(order) + skey

    # Dedupe (src -> dst-block): duplicate edges share one gathered row;
    # their one-hot row then carries multiple nonzeros (one per dst slot).
    osort = np.lexsort((row, key))
    keyo, rowo = key[osort], row[osort]
    lead = np.ones(ne, bool)
    lead[1:] = (keyo[1:] != keyo[:-1]) | (rowo[1:] != rowo[:-1])
    group = np.cumsum(lead) - 1                    # per sorted edge
    nl = int(lead.sum())
    lkey = keyo[lead]                              # bucket key per group

    cnt = np.bincount(lkey, minlength=C * len(order))
    cnt2 = cnt.reshape(C, len(order))
    # Unrounded bucket sizes (max unique count over cores).  Chunks of 128
    # edges may span bucket boundaries; only each (group, seg) call stream is
    # padded to a multiple of 128 (pad added to its last bucket).
    bsize = cnt2.max(axis=0).astype(np.int64)       # [n_buckets] stream order
    # stream boundaries: order is (g, s, blocks...) - find run ends
    stream_last = []
    i = 0
    for g in range(cfg.n_grp):
        nbs = len(range(g * GRP, min(cfg.n_blk, (g + 1) * GRP)))
        for s in range(cfg.n_seg):
            i += nbs
            stream_last.append(i - 1)
    run_start = 0
    for last in stream_last:
        tot = int(bsize[run_start:last + 1].sum())
        bsize[last] += (-tot) % 128
        run_start = last + 1
    seg_off = np.concatenate([[0], np.cumsum(bsize)])
    ep = int(seg_off[-1])

    starts = np.concatenate([[0], np.cumsum(cnt)])[:-1]
    rank = np.arange(nl) - starts[lkey]
    gpos = seg_off[lkey % len(order)] + rank       # slot per unique group
    pos = gpos[group]                              # per sorted edge
    core_s = keyo // len(order)

    idx16 = np.zeros((C, ep), np.int16)
    idx16[core_s[lead], gpos] = rowo[lead].astype(np.int16)
    nchunk = ep // 128
    # one-hot slabs: one [128, 128] slab per (bucket, chunk-it-overlaps),
    # laid out bucket-major in stream order
    c0b = seg_off[:-1] // 128                       # first chunk per bucket
    c1b = -(-(seg_off[:-1] + bsize) // 128)         # one-past-last chunk
    span = (c1b - c0b).astype(np.int64)
    slab_off = np.concatenate([[0], np.cumsum(span)])
    nslab = int(slab_off[-1])
    bkt_e = keyo % len(order)                       # bucket per sorted edge
    slab_e = slab_off[bkt_e] + (pos // 128 - c0b[bkt_e])
    oh = np.zeros((C, 128, nslab * 128), np.float32)
    np.add.at(oh, (core_s, pos % 128, slab_e * 128 + slot[osort]), ew[osort])
    oh = oh.astype(NP_FP8)

    idxw = idx16.reshape(C, ep // 16, 16).transpose(0, 2, 1)
    idxw = np.tile(idxw, (1, 8, 1)).copy()          # [C, 128, ep//16]

    # per-core aux arrays: gown0[c, l%128, (l//128)*128 + f] = g0[c*S+l, f]
    batch = np.asarray(batch).astype(np.int64)
    lr = np.arange(S)
    li = np.tile(lr, C)
    ci = np.repeat(np.arange(C), S)
    gown0 = np.zeros((C, 128, cfg.n_blk * 128), np.float32)
    gown0[ci[:, None], (li % 128)[:, None],
          ((li // 128) * 128)[:, None] + np.arange(cfg.d)[None, :]] = g0[
        ci * S + li]
    dinvb = np.zeros((C, 128, cfg.n_blk), np.float32)
    dinvb[ci, li % 128, li // 128] = dinv[ci * S + li]
    poolm = np.zeros((C, 128, cfg.n_blk * cfg.n_graphs), np.float32)
    poolm[ci, li % 128, (li // 128) * cfg.n_graphs + batch[ci * S + li]] = 1.0

    # layer-0 segment tables (padded local rows are zero)
    gfs0 = []
    for s in range(cfg.n_seg):
        rows = cfg.seg_rows[s]
        t = np.zeros((C * rows, cfg.d), np.float32)
        rs = lr[(lr >= cfg.seg_base[s]) & (lr < cfg.seg_end[s])]
        for c in range(C):
            t[c * rows + (rs - cfg.seg_base[s])] = g0[c * S + rs]
        gfs0.append(t.astype(NP_BF16))

    counts = np.bincount(batch, minlength=cfg.n_graphs).astype(np.float32)
    return dict(bsize=bsize, seg_off=seg_off, ep=ep, nchunk=nchunk,
                slab_off=slab_off, nslab=nslab,
                idxw=idxw, oh=oh,
                gown0=gown0.astype(NP_BF16), dinvb=dinvb,
                poolm=poolm.astype(NP_BF16), gfs0=gfs0,
                counts=counts)


def build_program(cfg, bsize, seg_off, slab_off, nslab, ep):
    """SPMD Bass/Tile program; trip counts depend only on bsize (shared)."""
    d, g64, n_blk = cfg.d, cfg.n_graphs, cfg.n_blk
    order = bucket_order(cfg)
    nchunk = ep // 128
    # per-bucket chunk spans and slab offsets (stream order)
    bidx = {order[i]: i for i in range(len(order))}
    ec0 = {order[i]: int(seg_off[i]) for i in range(len(order))}
    ec1 = {order[i]: int(seg_off[i] + bsize[i]) for i in range(len(order))}
    cc0 = {bh: e // 128 for bh, e in ec0.items()}
    cc1 = {bh: -(-e // 128) for bh, e in ec1.items()}
    sloff = {order[i]: int(slab_off[i]) for i in range(len(order))}

    nc = bacc.Bacc("TRN2", target_bir_lowering=False, debug=False,
                   enable_asserts=False, num_devices=cfg.n_cores,
                   num_swdge_queues=NQ)

    gfs_in = [nc.dram_tensor(f"gfs0_{s}",
                             [cfg.n_cores * cfg.seg_rows[s], d], BF16,
                             kind="ExternalInput") for s in range(cfg.n_seg)]
    gown_in = nc.dram_tensor("gown0", [128, n_blk * 128], BF16,
                             kind="ExternalInput")
    idx_in = nc.dram_tensor("idxw", [128, ep // 16], I16, kind="ExternalInput")
    oh_in = nc.dram_tensor("oh", [128, nslab * 128], FP8,
                           kind="ExternalInput")
    poolm_in = nc.dram_tensor("poolm", [128, n_blk * g64], BF16,
                              kind="ExternalInput")
    dinv_in = nc.dram_tensor("dinvb", [128, n_blk], F32, kind="ExternalInput")
    w_in = nc.dram_tensor("wmats", [3, d, d], BF16, kind="ExternalInput")
    b_in = nc.dram_tensor("biasb", [3, 128, d], F32, kind="ExternalInput")
    id_in = nc.dram_tensor("ident", [128, 128], BF16, kind="ExternalInput")
    out_t = nc.dram_tensor("pool_out", [g64, d], F32, kind="ExternalOutput")

    g_loc = [[nc.dram_tensor(f"g_loc{k}_{s}", [cfg.seg_rows[s], d], BF16,
                             kind="Internal") for s in range(cfg.n_seg)]
             for k in (1, 2)]
    gfs = [[nc.dram_tensor(f"gfs{k}_{s}",
                           [cfg.n_cores * cfg.seg_rows[s], d], BF16,
                           kind="Internal", addr_space="Shared")
            for s in range(cfg.n_seg)] for k in (1, 2)]
    rg = [list(range(cfg.n_cores))]
    # Collective schedule.  Segment s's AllGather must be EMITTED after the
    # block loop that writes its last g_loc rows; beyond that it is deferred
    # (emission point = after the NEXT group's seg-0 gather calls) so the
    # in-order Pool queue overlaps the semaphore wait with useful desc-gen.
    # Segments finishing in the final group are emitted inside the next
    # layer's first group (their data is only needed by its seg>=1 calls).
    ag_sched = {}   # group index -> segments, emitted before group's gathers
    ag_tail = []    # deferred into the next layer's group 0
    for s in range(cfg.n_seg):
        last_grp = cfg.seg_last_blk[s] // GRP
        if last_grp >= cfg.n_grp - 1:
            ag_tail.append(s)
        else:
            ag_sched.setdefault(min(last_grp + 2, cfg.n_grp - 1), []).append(s)

    with tile.TileContext(nc) as tc:
        with tc.tile_pool(name="const", bufs=1) as cp:
            idx_sb = cp.tile([128, ep // 16], I16, tag="idx")
            # split the idx load so the first group's gathers start sooner
            c_sp = max(8, (ep // 16) // 8)
            nc.sync.dma_start(idx_sb[:, :c_sp], idx_in.ap()[:, :c_sp])
            nc.sync.dma_start(idx_sb[:, c_sp:], idx_in.ap()[:, c_sp:])
            poolm_sb = cp.tile([128, n_blk * g64], BF16, tag="poolm")
            nc.sync.dma_start(poolm_sb[:, :], poolm_in.ap())
            dinv_sb = cp.tile([128, n_blk], F32, tag="dinv")
            nc.sync.dma_start(dinv_sb[:, :], dinv_in.ap())
            i128_sb = cp.tile([128, 128], BF16, tag="i128")
            nc.sync.dma_start(i128_sb[:, :], id_in.ap())
            w_sb, b_sb = [], []
            for k in range(3):
                wt = cp.tile([d, d], BF16, tag=f"w{k}", name=f"w{k}")
                nc.sync.dma_start(wt[:, :], w_in.ap()[k, :, :])
                w_sb.append(wt)
                bt = cp.tile([128, d], F32, tag=f"b{k}", name=f"b{k}")
                nc.sync.dma_start(bt[:, :], b_in.ap()[k, :, :])
                b_sb.append(bt)
            gpp = [[cp.tile([128, 128], BF16, tag=f"gown{i}_{b}",
                            name=f"gown{i}_{b}") for b in range(n_blk)]
                   for i in (0, 1)]
            for b in range(n_blk):
                nc.sync.dma_start(gpp[0][b][:, :],
                                  gown_in.ap()[:, b * 128:(b + 1) * 128])

            with tc.tile_pool(name="stage", bufs=30) as stp, \
                 tc.tile_pool(name="ohp", bufs=3) as ohp, \
                 tc.tile_pool(name="aggp", bufs=5, space="PSUM") as psA, \
                 tc.tile_pool(name="outp", bufs=2, space="PSUM") as psB, \
                 tc.tile_pool(name="poolp", bufs=1, space="PSUM") as psC, \
                 tc.tile_pool(name="work", bufs=6) as wp:
                qrr = [0]
                pp = None
                pending_ags = []
                for k in range(3):
                    tbl = [gfs_in[s].ap() if k == 0 else gfs[k - 1][s].ap()
                           for s in range(cfg.n_seg)]
                    gcur = gpp[k % 2]
                    gnxt = gpp[(k + 1) % 2]
                    for g in range(cfg.n_grp):
                        bs = range(g * GRP, min(n_blk, (g + 1) * GRP))
                        if k < 2:
                            for s in ag_sched.get(g, []):
                                nc.gpsimd.collective_compute(
                                    "AllGather", AluOpType.bypass,
                                    replica_groups=rg,
                                    ins=[g_loc[k][s].ap()],
                                    outs=[gfs[k][s].ap()])
                        chunkmap = {}
                        for h in range(cfg.n_seg):
                            c0 = cc0[(bs[0], h)]
                            c1 = ec1[(bs[-1], h)] // 128  # stream end aligned
                            base = tbl[h]
                            for cc in range(c0, c1, GMAX):
                                ncall = min(GMAX, c1 - cc)
                                st = stp.tile([128, ncall, 128], BF16,
                                              tag="st",
                                              name=f"st{k}_{g}_{h}_{cc}")
                                nc.gpsimd.dma_gather(
                                    st[:, :, :], base,
                                    idx_sb[:, cc * 8:(cc + ncall) * 8],
                                    ncall * 128, ncall * 128, d,
                                    queue_num=qrr[0] % NQ)
                                qrr[0] += 1
                                for j in range(ncall):
                                    chunkmap[cc + j] = (st, j)
                        for b in bs:
                            nch = sum(cc1[(b, h)] - cc0[(b, h)]
                                      for h in range(cfg.n_seg))
                            ohb = ohp.tile([128, nch * 128], FP8, tag="ohb",
                                           name=f"oh{k}_{b}")
                            o = 0
                            for h in range(cfg.n_seg):
                                nh = cc1[(b, h)] - cc0[(b, h)]
                                if nh == 0:
                                    continue
                                so = sloff[(b, h)]
                                nc.sync.dma_start(
                                    ohb[:, o * 128:(o + nh) * 128],
                                    oh_in.ap()[:, so * 128:(so + nh) * 128])
                                o += nh
                            pagg = psA.tile([128, 128], F32, tag="agg",
                                            name=f"agg{k}_{b}")
                            j = 0
                            for h in range(cfg.n_seg):
                                for i in range(cc1[(b, h)] - cc0[(b, h)]):
                                    st, jj = chunkmap[cc0[(b, h)] + i]
                                    nc.tensor.matmul(
                                        pagg[:, :], st[:, jj, :],
                                        ohb[:, j * 128:(j + 1) * 128],
                                        start=(j == 0), stop=False)
                                    j += 1
                            nc.tensor.matmul(
                                pagg[:, :], gcur[b][:, :],
                                i128_sb[:, :], start=(j == 0), stop=True)
                            aggT = wp.tile([128, 128], BF16, tag="aggT",
                                           name=f"aggT{k}_{b}")
                            nc.scalar.copy(aggT[:, :], pagg[:, :])
                            pout = psB.tile([128, d], F32, tag="out",
                                            name=f"out{k}_{b}")
                            nc.tensor.matmul(pout[:, :], aggT[:, :],
                                             w_sb[k][:, :], start=True,
                                             stop=True)
                            t2 = wp.tile([128, d], BF16, tag="t2",
                                         name=f"t2{k}_{b}")
                            nc.vector.scalar_tensor_tensor(
                                t2[:, :], pout[:, :], dinv_sb[:, b:b + 1],
                                b_sb[k][:, :], AluOpType.mult, AluOpType.add)
                            if k < 2:
                                # g_next = dinv*relu(t2) = relu(dinv*t2)
                                gt = gnxt[b]
                                nc.scalar.activation(
                                    gt[:, :], t2[:, :], AF.Relu,
                                    scale=dinv_sb[:, b:b + 1])
                                s = next(i for i in range(cfg.n_seg)
                                         if b * 128 < cfg.seg_end[i])
                                r0 = b * 128 - cfg.seg_base[s]
                                nc.sync.dma_start(
                                    g_loc[k][s].ap()[r0:r0 + 128, :],
                                    gt[:, :])
                            else:
                                if pp is None:
                                    pp = psC.tile([g64, d], F32, tag="pp")
                                nc.tensor.matmul(
                                    pp[:, :],
                                    poolm_sb[:, b * g64:(b + 1) * g64],
                                    t2[:, :], start=(b == 0),
                                    stop=(b == n_blk - 1))
                    if k < 2:
                        for s in ag_tail:
                            nc.gpsimd.collective_compute(
                                "AllGather", AluOpType.bypass,
                                replica_groups=rg,
                                ins=[g_loc[k][s].ap()],
                                outs=[gfs[k][s].ap()])
                ppsb = cp.tile([g64, d], F32, tag="ppsb")
                nc.scalar.copy(ppsb[:, :], pp[:, :])
                nc.sync.dma_start(out_t.ap(), ppsb[:, :])

    nc.compile()
    return nc


def make_in_maps(cfg, prep, ws, bs):
    wmats = np.stack([np.asarray(w, np.float32) for w in ws]).astype(NP_BF16)
    biasb = np.stack([np.broadcast_to(np.asarray(b, np.float32),
                                      (128, cfg.d)) for b in bs]).copy()
    ident = np.eye(128, dtype=np.float32).astype(NP_BF16)
    in_maps = []
    for c in range(cfg.n_cores):
        in_maps.append({
            **{f"gfs0_{s}": prep["gfs0"][s] for s in range(cfg.n_seg)},
            "gown0": prep["gown0"][c], "idxw": prep["idxw"][c],
            "oh": prep["oh"][c], "poolm": prep["poolm"][c],
            "dinvb": prep["dinvb"][c], "wmats": wmats, "biasb": biasb,
            "ident": ident,
        })
    return in_maps


_PROGRAM_CACHE = {}


def run(cfg, x, edge_index, edge_weight, batch, ws, bs, trace=False, trunc=""):
    prep = preprocess(cfg, edge_index, edge_weight, x, batch)
    key = (cfg.n_nodes, cfg.n_cores, prep["ep"], tuple(prep["bsize"]))
    nc = _PROGRAM_CACHE.get(key)
    if nc is None:
        nc = build_program(cfg, prep["bsize"], prep["seg_off"],
                           prep["slab_off"], prep["nslab"], prep["ep"])
        _PROGRAM_CACHE[key] = nc
    in_maps = make_in_maps(cfg, prep, ws, bs)
    res = bass_utils.run_bass_kernel_spmd(
        nc, in_maps, core_ids=list(range(cfg.n_cores)), trace=trace)
    partial = np.zeros((cfg.n_graphs, cfg.d), np.float64)
    for c in range(cfg.n_cores):
        partial += res.results[c]["pool_out"].astype(np.float64)
    out = (partial / np.maximum(prep["counts"], 1.0)[:, None]).astype(
        np.float32)
    return out, res


def kernel(x, edge_index, edge_weight, batch, W0, b0, W1, b1, W2, b2):
    cfg = Cfg()
    trace = bool(int(os.environ.get("GCN_TRACE", "0")))
    out, _ = run(cfg, x, edge_index, edge_weight, batch,
                 [W0, W1, W2], [b0, b1, b2], trace=trace)
    return out
